# revision 16
# baseline (speedup 1.0000x reference)
"""BondInfluenceSelfAttention TRN2 kernel (fp16 matmul datapath).

Full-input contract: kernel(**inputs) takes the complete unsharded inputs and
returns the full [B, L, D] output. Internally shards across 8 NeuronCores:
core c handles batch b = c // 4 and head-group g = c % 4 (4 heads, 256 dk
dims). Each core computes its heads' attention plus the partial output
projection through its 256 rows of Wo; the host sums the 4 partials per batch
and adds bo.

Device-side formulation (per core). All matmul operands are fp16; PSUM
accumulates fp32:
  KT = Wk_g^T x^T       [256, L]
  QT = (Wq_g/8)^T x^T   [256, L]   (1/sqrt(dk)=1/8 folded into Wq/bq on host)
  V  = x Wv_g           [L, 256]   (bias via an appended ones-row matmul; a
                                    ones column rides along so the softmax
                                    denominator accumulates in row 64)
  S^T tile = K Q^T      [L_k, L_q] per head (dk=64 contraction)
  P = exp(S^T * bondT)  bond multiply on DVE out of PSUM, exp on ACT over
                        2 L_k-tiles x 4 heads at once (N=4096 amortizes the
                        352-cycle ACT ramp), both fp16 out.
  1/denom = exp(-ln d) on ACT (vector.reciprocal is 8 cyc/elem on one lane;
  custom-DVE fast reciprocal misbehaves on this HW). The reciprocal row is
  replicated across 64 partitions by a partition-stride-0 SBUF->SBUF DMA,
  then DVE normalizes O^T out as fp16.
  Y = O Wo_g fp16 matmuls, fp32 out, summed across head-group cores on host.

Scheduling (the PE drops to its 1.2 GHz mid-pstate whenever the stream gaps,
vs 2.3 GHz sustained when saturated, so TensorE must never wait):
  - input DMAs fan out over sync/gpsimd/scalar queues, x^T k-tiles first;
  - preamble projections use 4 PSUM buffers so matmul bursts never wait on
    the PSUM->SBUF copies;
  - V-proj and next-chunk Q-proj matmuls interleave into the attention
    groups (chunk 0 carries V, every chunk carries the next chunk's Q);
  - AV matmuls trail the exp stage by two groups, and the output projection
    of chunk c is issued inside chunk c+1, so the in-order TensorE queue
    never parks behind DVE/ACT work at chunk boundaries.
"""

import numpy as np

try:
    import concourse.bass as bass  # noqa: F401
except ImportError:  # pragma: no cover
    import sys

    sys.path.insert(0, "/opt/trn_rl_repo")
    import concourse.bass as bass  # noqa: F401

import concourse.bacc as bacc
import concourse.mybir as mybir
import concourse.tile as tile
from concourse.bass_utils import run_bass_kernel_spmd

F32 = mybir.dt.float32
F16 = mybir.dt.float16

D = 1024  # d_model
L = 2048  # sequence length
B = 2  # batch
HPC = 4  # heads per core
DKG = 256  # dk dims per core (4 heads x 64)
NK = D // 128  # 8 contraction k-tiles for the projections
LT = L // 128  # 16 L-tiles
NCH = L // 512  # 4 L_q chunks
GRP = 2  # L_k tiles per exp group
NG = LT // GRP  # 8 groups per chunk
N_CORES = 8

_CACHED_NC = None


def _build_nc():
    nc = bacc.Bacc("TRN2", target_bir_lowering=False, debug=False, num_devices=N_CORES)

    xt_d = nc.declare_dram_parameter("xt", [D, L], F16, isOutput=False)
    bd_d = nc.declare_dram_parameter("bd", [L, L], F16, isOutput=False)
    wq_d = nc.declare_dram_parameter("wq", [D, DKG], F16, isOutput=False)
    wk_d = nc.declare_dram_parameter("wk", [D, DKG], F16, isOutput=False)
    wv_d = nc.declare_dram_parameter("wv", [D, DKG], F16, isOutput=False)
    bqk_d = nc.declare_dram_parameter("bqk", [128, 4], F32, isOutput=False)
    bv_d = nc.declare_dram_parameter("bv", [1, DKG], F16, isOutput=False)
    wo_d = nc.declare_dram_parameter("wo", [DKG, D], F16, isOutput=False)
    y_d = nc.declare_dram_parameter("y", [L, D], F32, isOutput=True)

    Exp = mybir.ActivationFunctionType.Exp
    Ln = mybir.ActivationFunctionType.Ln
    Identity = mybir.ActivationFunctionType.Identity

    with tile.TileContext(nc) as tc:
        with tc.tile_pool(name="persist", bufs=1) as pp:
            qt = [
                [
                    pp.tile([128, 512], F16, tag=f"qt{c}_{t}", name=f"qt{c}_{t}")
                    for t in range(2)
                ]
                for c in range(NCH)
            ]
            kt = [pp.tile([128, L], F16, tag=f"kt{t}", name=f"kt{t}") for t in range(2)]
            vt = [
                pp.tile([128, HPC, 66], F16, tag=f"v{i}", name=f"v{i}")
                for i in range(LT)
            ]
            ot = [pp.tile([128, L], F16, tag=f"ot{t}", name=f"ot{t}") for t in range(2)]
            wo_sb = pp.tile([128, 2, D], F16, tag="wo", name="wo_sb")
            bqk_sb = pp.tile([128, 4], F32, tag="bqk", name="bqk_sb")
            bv_sb = pp.tile([1, DKG], F16, tag="bv", name="bv_sb")
            ones_f = pp.tile([128, 128], F32, tag="onesf", name="ones_f")
            onesv = pp.tile([1, 128], F16, tag="onesv", name="onesv")
            onescol = pp.tile([128, HPC, 1], F32, tag="onescol", name="onescol")

            nc.vector.memset(ones_f, 1.0)
            nc.vector.tensor_copy(out=onesv, in_=ones_f[0:1, :])
            nc.vector.memset(onescol, 1.0)
            for i in range(LT):
                nc.vector.tensor_copy(out=vt[i][:, :, 64:65], in_=onescol)

            with tc.tile_pool(name="xw", bufs=1) as xw, tc.tile_pool(
                name="ps", bufs=1, space="PSUM"
            ) as ps, tc.tile_pool(name="att", bufs=1) as att:
                # x^T split per (chunk-column, k-tile) so the first K-proj
                # burst waits on 1 MB, not the whole 4 MB of x
                xkc = [
                    [
                        xw.tile([128, 512], F16, tag=f"x{c}_{k}", name=f"x{c}_{k}")
                        for k in range(NK)
                    ]
                    for c in range(NCH)
                ]
                wq_sb = xw.tile([128, NK, DKG], F16, tag="wq", name="wq_sb")
                wk_sb = xw.tile([128, NK, DKG], F16, tag="wk", name="wk_sb")
                wv_sb = xw.tile([128, NK, DKG], F16, tag="wv", name="wv_sb")

                xt_t = xt_d.ap().rearrange("(k p) (c l) -> k c p l", p=128, l=512)
                nc.gpsimd.dma_start(out=wk_sb, in_=wk_d.ap().rearrange("(k p) n -> p k n", p=128))
                nc.scalar.dma_start(out=bqk_sb, in_=bqk_d[:, :])
                queues = [nc.sync, nc.gpsimd, nc.scalar]
                qi = 0
                for c in range(NCH):
                    for k in range(NK):
                        queues[qi % 3].dma_start(out=xkc[c][k], in_=xt_t[k][c])
                        qi += 1
                    if c == 0:
                        nc.sync.dma_start(out=wq_sb, in_=wq_d.ap().rearrange("(k p) n -> p k n", p=128))
                    elif c == 1:
                        nc.gpsimd.dma_start(out=wv_sb, in_=wv_d.ap().rearrange("(k p) n -> p k n", p=128))
                        nc.scalar.dma_start(out=bv_sb, in_=bv_d[:, :])
                    elif c == 2:
                        nc.gpsimd.dma_start(out=wo_sb, in_=wo_d.ap().rearrange("(t p) n -> p t n", p=128))

                bd_g = bd_d.ap().rearrange("(g t p) l -> g p t l", p=128, t=GRP)

                def kproj(t, c, tag):
                    pk = ps.tile([128, 512], F32, tag=tag, bufs=4 if tag == "oacc" else 2, name="pk")
                    for k in range(NK):
                        nc.tensor.matmul(
                            pk[:, :],
                            wk_sb[:, k, 128 * t : 128 * (t + 1)],
                            xkc[c][k][:, :],
                            start=(k == 0),
                            stop=(k == NK - 1),
                        )
                    nc.scalar.activation(
                        out=kt[t][:, 512 * c : 512 * (c + 1)],
                        in_=pk[:, :],
                        func=Identity,
                        bias=bqk_sb[:, 2 + t : 3 + t],
                    )

                def qproj(t, c, tag):
                    pq = ps.tile([128, 512], F32, tag=tag, bufs=4 if tag == "oacc" else 2, name="pq")
                    for k in range(NK):
                        nc.tensor.matmul(
                            pq[:, :],
                            wq_sb[:, k, 128 * t : 128 * (t + 1)],
                            xkc[c][k][:, :],
                            start=(k == 0),
                            stop=(k == NK - 1),
                        )
                    nc.scalar.activation(
                        out=qt[c][t][:, :],
                        in_=pq[:, :],
                        func=Identity,
                        bias=bqk_sb[:, t : t + 1],
                    )

                def vproj(i):
                    pv = ps.tile([128, DKG], F32, tag="s", bufs=2, name="pv")
                    for k in range(NK):
                        nc.tensor.matmul(
                            pv[:, :],
                            xkc[i // 4][k][:, 128 * (i % 4) : 128 * (i % 4 + 1)],
                            wv_sb[:, k, :],
                            start=(k == 0),
                            stop=False,
                        )
                    nc.tensor.matmul(
                        pv[:, :], onesv[:, :], bv_sb[:, :], start=False, stop=True
                    )
                    nc.scalar.activation(
                        out=vt[i][:, :, 0:64],
                        in_=pv.rearrange("p (h e) -> p h e", e=64),
                        func=Identity,
                    )

                # ---------------- preamble: K (all chunks) + Q (chunk 0) ----
                for t in range(2):
                    for c in range(NCH):
                        kproj(t, c, "oacc")
                for t in range(2):
                    qproj(t, 0, "oacc")

                # ---------------- attention chunks --------------------------
                pend_yproj = None  # (yp tiles of prev chunk, chunk index)
                pend_norm = None  # (reciprocal tile, oaccs, chunk index)

                def issue_yproj(pend):
                    yps, cprev = pend
                    for idx, (j, dh) in enumerate(
                        (j, dh) for j in range(4 * cprev, 4 * cprev + 4) for dh in range(2)
                    ):
                        yp = yps[idx]
                        for t in range(2):
                            nc.tensor.matmul(
                                yp[:, :],
                                ot[t][:, 128 * j : 128 * (j + 1)],
                                wo_sb[:, t, 512 * dh : 512 * (dh + 1)],
                                start=(t == 0),
                                stop=(t == 1),
                            )
                        ys = att.tile([128, 512], F32, tag="ys", bufs=4, name="ys")
                        if dh == 0:
                            nc.scalar.activation(out=ys, in_=yp[:, :], func=Identity)
                        else:
                            nc.vector.tensor_copy(out=ys, in_=yp[:, :])
                        nc.gpsimd.dma_start(
                            out=y_d[128 * j : 128 * (j + 1), 512 * dh : 512 * (dh + 1)],
                            in_=ys,
                        )

                def issue_recip(oaccs_p):
                    # 1/denom = exp(-ln d) on ACT; the denominators sit on
                    # partition 64 of each accumulator
                    ld = att.tile([65, HPC, 512], F32, tag="ld", bufs=2, name="ld")
                    rd = att.tile([65, HPC, 512], F16, tag="rd", bufs=2, name="rd")
                    for h in range(HPC):
                        nc.scalar.activation(
                            out=ld[64:65, h, :], in_=oaccs_p[h][64:65, :], func=Ln
                        )
                    nc.scalar.activation(
                        out=rd[64:65, :, :], in_=ld[64:65, :, :], func=Exp, scale=-1.0
                    )
                    return rd

                def issue_norm(rd, oaccs_p, cprev):
                    # replicate each reciprocal row across 64 partitions with
                    # a step-0 free-dim DMA, then scale O^T out as fp16
                    for t in range(2):
                        for half in range(2):
                            h = 2 * t + half
                            rr = rd[64:65, h, :]
                            rr_bcast = bass.AP(
                                tensor=rr.tensor,
                                offset=rr.offset,
                                ap=[rr.ap[0], [0, 64]] + list(rr.ap[1:]),
                            )
                            bcs = att.tile([64, 512], F16, tag="bcs", bufs=3, name="bcs")
                            nc.sync.dma_start(out=bcs, in_=rr_bcast)
                            if half == 0:
                                nc.vector.tensor_mul(
                                    out=ot[t][0:64, 512 * cprev : 512 * (cprev + 1)],
                                    in0=oaccs_p[h][0:64, :],
                                    in1=bcs,
                                )
                            else:
                                odd = att.tile([64, 512], F16, tag="odd", bufs=2, name="odd")
                                nc.vector.tensor_mul(
                                    out=odd,
                                    in0=oaccs_p[h][0:64, :],
                                    in1=bcs,
                                )
                                nc.gpsimd.dma_start(
                                    out=ot[t][64:128, 512 * cprev : 512 * (cprev + 1)],
                                    in_=odd,
                                )

                for c in range(NCH):
                    oaccs = [
                        ps.tile([128, 512], F32, tag="oacc", bufs=4, name=f"oacc{h}")
                        for h in range(HPC)
                    ]
                    if pend_norm is not None:
                        issue_norm(*pend_norm)
                        pend_norm = None
                    pend = []  # delayed AV issue: (pt tile, group index)

                    def flush_av(pt_g, g):
                        for ii in range(GRP):
                            i = GRP * g + ii
                            for h in range(HPC):
                                nc.tensor.matmul(
                                    oaccs[h][0:65, :],
                                    vt[i][:, h, 0:65],
                                    pt_g[:, ii, h, :],
                                    start=(i == 0),
                                    stop=(i == LT - 1),
                                )

                    for g in range(NG):
                        prod = att.tile(
                            [128, GRP, HPC, 512], F16, tag="prod", bufs=3, name="prod"
                        )
                        pt_g = att.tile(
                            [128, GRP, HPC, 512], F16, tag="pt", bufs=3, name="pt"
                        )
                        bt2 = att.tile([128, GRP, 512], F16, tag="bond", bufs=3, name="bt2")
                        nc.sync.dma_start(
                            out=bt2,
                            in_=bd_g[g][:, :, 512 * c : 512 * (c + 1)],
                        )
                        for ii in range(GRP):
                            i = GRP * g + ii
                            if c == 0:
                                vproj(i)
                                if i == 4:
                                    qproj(0, 1, "s")
                                elif i == 10:
                                    qproj(1, 1, "s")
                            elif c < NCH - 1:
                                if i == 4:
                                    qproj(0, c + 1, "s")
                                elif i == 10:
                                    qproj(1, c + 1, "s")
                            bt = bt2[:, ii, :]
                            bt_bcast = bass.AP(
                                tensor=bt.tensor,
                                offset=bt.offset,
                                ap=[bt.ap[0], [0, 2]] + list(bt.ap[1:]),
                            )
                            for t in range(2):
                                spair = ps.tile(
                                    [128, 2, 512], F32, tag="s", bufs=2, name="spair"
                                )
                                for half in range(2):
                                    nc.tensor.matmul(
                                        spair[:, half, :],
                                        kt[t][64 * half : 64 * (half + 1), 128 * i : 128 * (i + 1)],
                                        qt[c][t][64 * half : 64 * (half + 1), :],
                                        start=True,
                                        stop=True,
                                    )
                                nc.vector.tensor_mul(
                                    out=prod[:, ii, 2 * t : 2 * (t + 1), :],
                                    in0=spair,
                                    in1=bt_bcast,
                                )
                        nc.scalar.activation(out=pt_g, in_=prod, func=Exp)
                        if g == 1 and pend_yproj is not None:
                            issue_yproj(pend_yproj)
                            pend_yproj = None
                        pend.append((pt_g, g))
                        if len(pend) > 2:
                            flush_av(*pend.pop(0))
                    for pt_g, g in pend:
                        flush_av(pt_g, g)

                    # reciprocal now (ACT is otherwise drained at chunk end);
                    # normalize + output projection are issued inside the next
                    # chunk so neither the DVE nor the TensorE queue parks at
                    # the boundary -- except for the last chunk
                    rd = issue_recip(oaccs)
                    yps = [
                        ps.tile([128, 512], F32, tag="oacc", bufs=4, name="yp")
                        for _ in range(8)
                    ]
                    pend_norm = (rd, oaccs, c)
                    pend_yproj = (yps, c)
                    if c == NCH - 1:
                        issue_norm(*pend_norm)
                        pend_norm = None
                        issue_yproj(pend_yproj)
                        pend_yproj = None

    nc.compile()
    return nc


def _get_nc():
    global _CACHED_NC
    if _CACHED_NC is None:
        _CACHED_NC = _build_nc()
    return _CACHED_NC


def _host_prep(x, bond_influence, Wq, bq, Wk, bk, Wv, bv, Wo):
    xt_b = [np.ascontiguousarray(x[b].T.astype(np.float16)) for b in range(B)]
    bd_b = [
        np.ascontiguousarray(bond_influence[b].T.astype(np.float16)) for b in range(B)
    ]
    in_maps = []
    for core in range(N_CORES):
        b, g = core // HPC, core % HPC
        s = slice(g * DKG, (g + 1) * DKG)
        bq_g = (bq[s] / 8.0).astype(np.float32)
        bk_g = bk[s].astype(np.float32)
        bqk = np.stack(
            [bq_g[0:128], bq_g[128:256], bk_g[0:128], bk_g[128:256]], axis=1
        )
        in_maps.append(
            {
                "xt": xt_b[b],
                "bd": bd_b[b],
                "wq": np.ascontiguousarray((Wq[:, s] / 8.0).astype(np.float16)),
                "wk": np.ascontiguousarray(Wk[:, s].astype(np.float16)),
                "wv": np.ascontiguousarray(Wv[:, s].astype(np.float16)),
                "bqk": np.ascontiguousarray(bqk),
                "bv": np.ascontiguousarray(bv[s][None, :].astype(np.float16)),
                "wo": np.ascontiguousarray(Wo[s, :].astype(np.float16)),
            }
        )
    return in_maps


def kernel(
    x,
    bond_influence,
    Wq,
    bq,
    Wk,
    bk,
    Wv,
    bv,
    Wo,
    bo,
    _trace=False,
    _trace_out=None,
):
    x = np.asarray(x, dtype=np.float32)
    bond_influence = np.asarray(bond_influence, dtype=np.float32)
    args = [np.asarray(a, dtype=np.float32) for a in (Wq, bq, Wk, bk, Wv, bv, Wo)]
    bo = np.asarray(bo, dtype=np.float32)

    nc = _get_nc()
    in_maps = _host_prep(x, bond_influence, *args)
    kwargs = {}
    if _trace:
        kwargs = dict(trace=True, tmpdir=_trace_out)
    res = run_bass_kernel_spmd(nc, in_maps, list(range(N_CORES)), **kwargs)

    out = np.zeros((B, L, D), dtype=np.float32)
    for b in range(B):
        acc = res.results[4 * b]["y"].astype(np.float32).copy()
        for g in range(1, HPC):
            acc += res.results[4 * b + g]["y"]
        out[b] = acc + bo[None, :]
    if _trace:
        return out, res
    return out
